# revision 1
# baseline (speedup 1.0000x reference)
"""Trainium2 Bass kernel for nn_Attention_5935644803277 (CvT-style sparse attention).

Full-input contract: kernel(**inputs) takes the unsharded inputs (x: [32,1536,768])
and returns the full output [32,1536,768]. Internally shards batch 32 -> 4 per core
across 8 NeuronCores (SPMD, no collectives).

Math (per batch):
  tpl = x[:256] as 16x16 image, onl = x[256:512] as 16x16, srch = x[512:] as 32x32
  q = concat(dwconv3x3_s1(img) for img) -> BN -> @ wq.T   (1536 tokens)
  k,v = same with stride 2 -> 384 tokens
  heads(12, hd=64); templates (first 512 q) attend to first 128 k/v;
  search (last 1024 q) attend to all 384; softmax(QK^T * 768^-0.5);
  out = concat @ w_proj.T + b_proj

Kernel-side simplifications (host-precomputed):
  - BN folded into projection weights: W*_eff[c,d] = w*[d,c] * inv*[c]
  - K-projection bias dropped (softmax shift invariance)
  - V-projection bias folded into final bias: b_fin = b_proj + w_proj @ (wv @ beta_v)
  - Q-projection bias bq_eff = wq @ beta_q applied at PSUM evacuation
  - softmax denominator via ones-column appended to V stationary
"""
import numpy as np

import concourse.bass as bass
import concourse.tile as tile
from concourse import bacc, mybir
from concourse.bass_utils import run_bass_kernel_spmd

F32 = mybir.dt.float32
BF16 = mybir.dt.bfloat16
AF = mybir.ActivationFunctionType
OP = mybir.AluOpType

EPS = 1e-5
NB = 4          # batches per core
L = 1536
D = 768
G = 6           # channel chunks of 128
NH = 12
HD = 64
SCALE = float(D) ** -0.5
LKV = 384


def _rect(tile_ap, base, dims):
    """AP at tile's partition dim + given free-dim [step,count] list, at free offset base."""
    return bass.AP(tensor=tile_ap.tensor, offset=tile_ap.offset + base,
                   ap=[list(tile_ap.ap[0])] + [list(d) for d in dims])


def _tap_bounds(d, H):
    """stride-1 output row range for tap offset d in {0,1,2}."""
    r0 = 1 if d == 0 else 0
    r1 = H - 2 if d == 2 else H - 1
    return r0, r1 - r0 + 1


def _tap_bounds_s2(d, H):
    """stride-2: output rows where input row 2r+d-1 in [0,H). H even."""
    Ho = H // 2
    r0 = 1 if d == 0 else 0
    return r0, Ho - r0


def build_program(skip=(), nb=NB, reps=1):
    nc = bacc.Bacc("TRN2", target_bir_lowering=False, debug=False, num_devices=8)

    x_d = nc.dram_tensor("x", [nb, L, D], F32, kind="ExternalInput").ap()
    w_d = {n: nc.dram_tensor(n, [D, D], F32, kind="ExternalInput").ap()
           for n in ("wq", "wk", "wv", "wp")}
    bq_d = nc.dram_tensor("bq", [128, G], F32, kind="ExternalInput").ap()
    bfin_d = nc.dram_tensor("bfin", [1, D], F32, kind="ExternalInput").ap()
    cw_d = {n: nc.dram_tensor(n, [128, G, 9], F32, kind="ExternalInput").ap()
            for n in ("cwq", "cwk", "cwv")}
    ident_d = nc.dram_tensor("ident", [128, 128], F32, kind="ExternalInput").ap()
    diags_d = nc.dram_tensor("diags", [2, G, 9, 128, 128], BF16, kind="ExternalInput").ap()
    ones12_d = nc.dram_tensor("ones12", [128, NH], F32, kind="ExternalInput").ap()
    ones1_d = nc.dram_tensor("ones1", [1, 128], F32, kind="ExternalInput").ap()
    out_d = nc.dram_tensor("out", [nb, L, D], F32, kind="ExternalOutput").ap()

    with tile.TileContext(nc) as tc:
        with (
            tc.tile_pool(name="consts", bufs=1) as consts,
            tc.tile_pool(name="wpool", bufs=1) as wpool,
            tc.tile_pool(name="act", bufs=1) as actp,
            tc.tile_pool(name="roll", bufs=1) as roll,
            tc.tile_pool(name="ps_big", bufs=2, space="PSUM") as ps_big,   # score pairs [128,1024]
            tc.tile_pool(name="ps_pj", bufs=2, space="PSUM") as ps_pj,     # projections [128,512]
            tc.tile_pool(name="ps_u", bufs=2, space="PSUM") as ps_u,       # attention A@V+sums
            tc.tile_pool(name="dram", bufs=2, space="DRAM") as dramp,
        ):
            # ---------------- constants ----------------
            ident_sb = consts.tile([128, 128], F32, name="ident_sb")
            nc.sync.dma_start(out=ident_sb, in_=ident_d)
            bq_sb = consts.tile([128, G], F32, name="bq_sb")
            nc.sync.dma_start(out=bq_sb, in_=bq_d)
            ones12_sb = consts.tile([128, NH], F32, name="ones12_sb")
            nc.sync.dma_start(out=ones12_sb, in_=ones12_d)
            bfin_f = consts.tile([1, D], F32, name="bfin_f")
            nc.sync.dma_start(out=bfin_f, in_=bfin_d)
            bfin_sb = consts.tile([1, D], BF16, name="bfin_sb")
            nc.scalar.copy(bfin_sb, bfin_f)
            ones1_f = consts.tile([1, 128], F32, name="ones1_f")
            nc.sync.dma_start(out=ones1_f, in_=ones1_d)
            ones1_sb = consts.tile([1, 128], BF16, name="ones1_sb")
            nc.scalar.copy(ones1_sb, ones1_f)
            ones768_sb = consts.tile([128, D], F32, name="ones768_sb")
            nc.vector.memset(ones768_sb, 1.0)
            cw_sb = {}
            for n in ("cwq", "cwk", "cwv"):
                cw_sb[n] = consts.tile([128, G, 9], F32, name=f"{n}_sb")
                nc.sync.dma_start(out=cw_sb[n], in_=cw_d[n])
            w_sb = {}
            for n in ("wq", "wk", "wv", "wp"):
                w_sb[n] = wpool.tile([128, G, D], BF16, name=f"{n}_sb")
                for g in range(G):
                    wst = roll.tile([128, D], F32, name=f"wst_{n}_{g}",
                                    tag="wst", bufs=1)
                    nc.sync.dma_start(out=wst, in_=w_d[n][g * 128:(g + 1) * 128, :])
                    nc.scalar.copy(w_sb[n][:, g, :], wst)

            # ------------- helpers (closures) -------------
            def conv_q_taps(cq, xt, g, img_bases, H, tag, row_lo=0, row_hi=None):
                """8 non-center taps over one or two HxH images (stride 1),
                output rows [row_lo, row_hi). Each tap: packed product on a tmp
                tile (tensor_scalar, 4x DVE mode) + accumulate add
                (tensor_tensor, 2x) — scalar_tensor_tensor would run at 1x."""
                if row_hi is None:
                    row_hi = H
                cwq = cw_sb["cwq"]
                gb = g * L
                nimg = len(img_bases)
                istep = img_bases[1] - img_bases[0] if nimg == 2 else 0
                for dh in range(3):
                    for dw in range(3):
                        if dh == 1 and dw == 1:
                            continue
                        sc = cwq[:, g, 3 * dh + dw:3 * dh + dw + 1]
                        r0, nr = _tap_bounds(dh, H)
                        c0, nw = _tap_bounds(dw, H)
                        rr1 = min(r0 + nr - 1, row_hi - 1)
                        r0 = max(r0, row_lo)
                        nr = rr1 - r0 + 1
                        if nr <= 0:
                            continue
                        for ib in img_bases:
                            tmp = roll.tile([128, 1024], BF16,
                                            name=f"tmp_{tag}_{dh}{dw}",
                                            tag="cvtmp", bufs=6)
                            ia = _rect(xt, gb + ib + (r0 + dh - 1) * H + (c0 + dw - 1),
                                       [[H, nr], [1, nw]])
                            ta = _rect(tmp, 0, [[nw, nr], [1, nw]])
                            nc.vector.tensor_scalar_mul(ta, ia, sc)
                            oa = _rect(cq, gb + ib + r0 * H + c0,
                                       [[H, nr], [1, nw]])
                            nc.vector.tensor_tensor(
                                out=oa, in0=oa, in1=ta, op=OP.add)

            # ---------------- per batch ----------------
            for rep_i, b in enumerate(bb for _ in range(reps) for bb in range(nb)):
                rb = f"{rep_i}"
                # ---- x[b] -> bf16 (gpsimd cast DMA) -> transpose via HWDGE
                # DMA-transpose into xt (channels on partitions); no PE/ACT work
                xt = actp.tile([128, G, L], BF16, name=f"xt_{rb}", tag="xt", bufs=2)
                xbf = dramp.tile([L, D], BF16, name=f"xbf_{rb}", tag="xbf", bufs=2)
                nc.gpsimd.dma_start(out=xbf, in_=x_d[b])
                for g in range(G):
                    nc.sync.dma_start_transpose(
                        xt[:, g, :], xbf[:, g * 128:(g + 1) * 128])

                # ---- conv-q on tpl+onl tokens (feeds attention t=0)
                cq = actp.tile([128, G, L], BF16, name=f"cq_{rb}", tag="cq", bufs=2)
                cwq = cw_sb["cwq"]
                for g in range(G):
                    nc.vector.tensor_scalar_mul(
                        cq[:, g, 0:512], xt[:, g, 0:512], cwq[:, g, 4:5])
                    conv_q_taps(cq, xt, g, (0, 256), 16, f"to_{b}_{g}")

                # ---- conv-k/v on PE via diag matmuls
                ckv = actp.tile([128, 2, G, LKV], BF16, name=f"ckv_{rb}", tag="ckv")
                for ci in range(2):
                    for g in range(G):
                        dgt = roll.tile([128, 9, 128], BF16,
                                        name=f"dg_{rb}_{ci}_{g}", tag="dgs", bufs=2)
                        nc.sync.dma_start(
                            out=dgt,
                            in_=diags_d[ci, g].rearrange("t p m -> p t m"))
                        kvp = ps_pj.tile([128, LKV], F32,
                                         name=f"kvp_{rb}_{ci}_{g}", tag="pj")
                        gb = g * L
                        taps = [(1, 1)] + [(dh, dw) for dh in range(3)
                                           for dw in range(3) if (dh, dw) != (1, 1)]
                        n_mm = 2 * len(taps)
                        mm_i = 0
                        for dh, dw in taps:
                            dg = dgt[:, 3 * dh + dw, :]
                            r0, nr = _tap_bounds_s2(dh, 16)
                            c0, nw = _tap_bounds_s2(dw, 16)
                            ia = _rect(xt, gb + (2 * r0 + dh - 1) * 16 + (2 * c0 + dw - 1),
                                       [[256, 2], [32, nr], [2, nw]])
                            oa = _rect(kvp, r0 * 8 + c0, [[64, 2], [8, nr], [1, nw]])
                            nc.tensor.matmul(oa, dg, ia, start=(mm_i == 0),
                                             stop=(mm_i == n_mm - 1),
                                             skip_group_check=True)
                            mm_i += 1
                            r0, nr = _tap_bounds_s2(dh, 32)
                            c0, nw = _tap_bounds_s2(dw, 32)
                            ia = _rect(xt, gb + 512 + (2 * r0 + dh - 1) * 32 + (2 * c0 + dw - 1),
                                       [[64, nr], [2, nw]])
                            oa = _rect(kvp, 128 + r0 * 16 + c0, [[16, nr], [1, nw]])
                            nc.tensor.matmul(oa, dg, ia, start=False,
                                             stop=(mm_i == n_mm - 1),
                                             skip_group_check=True)
                            mm_i += 1
                        nc.scalar.copy(ckv[:, ci, g, :], kvp)

                # ---- K projection (transposed layout)
                kt = actp.tile([128, G, LKV], BF16, name=f"kt_{rb}", tag="kt")
                for go in range(G):
                    pj = ps_pj.tile([128, LKV], F32, name=f"kpj_{rb}_{go}", tag="pj")
                    for g in range(G):
                        nc.tensor.matmul(
                            pj, w_sb["wk"][:, g, go * 128:(go + 1) * 128],
                            ckv[:, 0, g, :], start=(g == 0), stop=(g == G - 1))
                    nc.scalar.copy(kt[:, go, :], pj)

                # ---- V projection (natural) + 64-wide ones block
                v_sb = actp.tile([128, 3, NH, 2 * HD], BF16, name=f"v_{rb}", tag="v")
                for mt in range(3):
                    oa = _rect(v_sb, mt * NH * 2 * HD + HD, [[2 * HD, NH], [1, HD]])
                    nc.scalar.copy(oa, ones768_sb)
                    for nh in range(2):
                        pj = ps_pj.tile([128, LKV], F32,
                                        name=f"vpj_{rb}_{mt}_{nh}", tag="pj")
                        for g in range(G):
                            nc.tensor.matmul(
                                pj, ckv[:, 1, g, mt * 128:(mt + 1) * 128],
                                w_sb["wv"][:, g, nh * 384:(nh + 1) * 384],
                                start=(g == 0), stop=(g == G - 1))
                        oa = _rect(v_sb, mt * NH * 2 * HD + nh * 6 * 2 * HD,
                                   [[2 * HD, 6], [1, HD]])
                        nc.scalar.copy(oa, pj)

                # ---- attention, one 512-query tile at a time; the search-token
                #      conv halves are emitted between tiles so DVE conv overlaps
                #      PE attention
                for t in range(3):
                    if t > 0:
                        # conv-q for srch rows [16(t-1), 16t) — overlaps the
                        # previous tile's PE attention work
                        for g in range(G):
                            lo = 512 + 512 * (t - 1)
                            nc.vector.tensor_scalar_mul(
                                cq[:, g, lo:lo + 512], xt[:, g, lo:lo + 512],
                                cwq[:, g, 4:5])
                            conv_q_taps(cq, xt, g, (512,), 32, f"s_{b}_{t}_{g}",
                                        row_lo=16 * (t - 1), row_hi=16 * t)

                    qt = roll.tile([128, G, 512], BF16, name=f"qt_{rb}_{t}",
                                   tag="qt", bufs=2)
                    for go in range(G):
                        pj = ps_pj.tile([128, 512], F32,
                                        name=f"qpj_{rb}_{t}_{go}", tag="pj")
                        for g in range(G):
                            nc.tensor.matmul(
                                pj, w_sb["wq"][:, g, go * 128:(go + 1) * 128],
                                cq[:, g, t * 512:(t + 1) * 512],
                                start=(g == 0), stop=(g == G - 1))
                        nc.scalar.activation(qt[:, go, :], pj, AF.Identity,
                                             bias=bq_sb[:, go:go + 1])

                    xatt = roll.tile([128, G, 512], BF16, name=f"xatt_{rb}_{t}",
                                     tag="xatt", bufs=2)
                    kcs = (0,) if t == 0 else (0, 1, 2)
                    for hp in range(6):   # head pairs (2hp, 2hp+1), same g=hp
                        g = hp
                        ups = [ps_u.tile([128, 512], F32,
                                         name=f"u_{rb}_{t}_{hp}_{j}", tag="u")
                               for j in range(2)]
                        for i, kc in enumerate(kcs):
                            sps = ps_big.tile([128, 1024], F32,
                                              name=f"s_{rb}_{t}_{hp}_{kc}", tag="big")
                            # two heads' score chunks in one tile; row bases 0/64
                            # target distinct PE quadrants -> concurrent matmuls
                            for j, po in enumerate((0, HD)):
                                nc.tensor.matmul(
                                    sps[:, j * 512:(j + 1) * 512],
                                    kt[po:po + HD, g, kc * 128:(kc + 1) * 128],
                                    qt[po:po + HD, g, :], start=True, stop=True)
                            aT = roll.tile([128, 1024], BF16,
                                           name=f"aT_{rb}_{t}_{hp}_{kc}",
                                           tag="aT", bufs=4)
                            nc.scalar.activation(aT, sps, AF.Exp, scale=SCALE)
                            for j in range(2):
                                nc.tensor.matmul(
                                    ups[j], v_sb[:, kc, 2 * hp + j, :],
                                    aT[:, j * 512:(j + 1) * 512],
                                    start=(i == 0), stop=(i == len(kcs) - 1))
                        for j in range(2):
                            recip = roll.tile([HD, 512], F32,
                                              name=f"rc_{rb}_{t}_{hp}_{j}",
                                              tag="recip", bufs=3)
                            nc.vector.reciprocal(recip, ups[j][HD:2 * HD, :])
                            nc.vector.tensor_mul(
                                xatt[j * HD:(j + 1) * HD, g, :],
                                ups[j][0:HD, :], recip)

                    # ---- output projection (natural layout) + bias + store
                    for mt2 in range(4):
                        onat = roll.tile([128, D], F32, name=f"on_{rb}_{t}_{mt2}",
                                         tag="onat", bufs=3)
                        for nh in range(2):
                            pj = ps_big.tile([128, 384], F32,
                                             name=f"opj_{rb}_{t}_{mt2}_{nh}", tag="big")
                            for g in range(G):
                                nc.tensor.matmul(
                                    pj, xatt[:, g, mt2 * 128:(mt2 + 1) * 128],
                                    w_sb["wp"][:, g, nh * 384:(nh + 1) * 384],
                                    start=(g == 0), stop=False)
                            nc.tensor.matmul(
                                pj, ones1_sb, bfin_sb[:, nh * 384:(nh + 1) * 384],
                                start=False, stop=True)
                            nc.scalar.copy(onat[:, nh * 384:(nh + 1) * 384], pj)
                        tok0 = t * 512 + mt2 * 128
                        nc.sync.dma_start(out=out_d[b, tok0:tok0 + 128, :], in_=onat)

    nc.compile()
    return nc


_NC_CACHE = {}


def _get_program():
    if "nc" not in _NC_CACHE:
        _NC_CACHE["nc"] = build_program()
    return _NC_CACHE["nc"]


def _host_prep(inputs):
    f = lambda k: np.asarray(inputs[k], dtype=np.float32)
    w = {}
    effs = {}
    for n in ("q", "k", "v"):
        inv = f(f"bn_{n}_g") / np.sqrt(f(f"bn_{n}_v") + EPS)
        beta = f(f"bn_{n}_b") - f(f"bn_{n}_m") * inv
        effs[n] = (inv, beta)
    wq, wk, wv, wp = f("wq"), f("wk"), f("wv"), f("w_proj")
    w["wq"] = np.ascontiguousarray((wq * effs["q"][0][None, :]).T)
    w["wk"] = np.ascontiguousarray((wk * effs["k"][0][None, :]).T)
    w["wv"] = np.ascontiguousarray((wv * effs["v"][0][None, :]).T)
    w["wp"] = np.ascontiguousarray(wp.T)
    bq_eff = wq @ effs["q"][1]
    bv_eff = wv @ effs["v"][1]
    b_fin = f("b_proj") + wp @ bv_eff
    w["bq"] = np.ascontiguousarray(bq_eff.reshape(G, 128).T)
    w["bfin"] = b_fin.reshape(1, D)
    for n, key in (("cwq", "conv_q_w"), ("cwk", "conv_k_w"), ("cwv", "conv_v_w")):
        cw = f(key).reshape(D, 9)
        w[n] = np.ascontiguousarray(cw.reshape(G, 128, 9).transpose(1, 0, 2))
    w["ident"] = np.eye(128, dtype=np.float32)
    import ml_dtypes
    eye = np.eye(128, dtype=np.float32)
    dg = np.zeros((2, G, 9, 128, 128), np.float32)
    for ci, n in enumerate(("cwk", "cwv")):
        cw = w[n]  # [128, G, 9]
        for g in range(G):
            for t in range(9):
                dg[ci, g, t] = eye * cw[:, g, t][:, None]
    w["diags"] = dg.astype(ml_dtypes.bfloat16)
    w["ones12"] = np.ones((128, NH), np.float32)
    w["ones1"] = np.ones((1, 128), np.float32)
    return {k: (np.ascontiguousarray(v) if k == "diags"
                else np.ascontiguousarray(v, dtype=np.float32))
            for k, v in w.items()}


def kernel(**inputs):
    x = np.asarray(inputs["x"], dtype=np.float32)
    B = x.shape[0]
    assert x.shape == (32, L, D), x.shape
    const = _host_prep(inputs)
    nc = _get_program()
    in_maps = []
    for c in range(8):
        m = dict(const)
        m["x"] = np.ascontiguousarray(x[c * NB:(c + 1) * NB])
        in_maps.append(m)
    res = run_bass_kernel_spmd(nc, in_maps, list(range(8)))
    out = np.concatenate([res.results[c]["out"] for c in range(8)], axis=0)
    return out.astype(np.float32)



# revision 14
# speedup vs baseline: 1.0433x; 1.0433x over previous
"""Trainium2 Bass kernel for nn_Attention_5935644803277 (CvT-style sparse attention).

Full-input contract: kernel(**inputs) takes the unsharded inputs (x: [32,1536,768])
and returns the full output [32,1536,768]. Internally shards batch 32 -> 4 per core
across 8 NeuronCores (SPMD, no collectives).

Math (per batch):
  tpl = x[:256] as 16x16 image, onl = x[256:512] as 16x16, srch = x[512:] as 32x32
  q = concat(dwconv3x3_s1(img) for img) -> BN -> @ wq.T   (1536 tokens)
  k,v = same with stride 2 -> 384 tokens
  heads(12, hd=64); templates (first 512 q) attend to first 128 k/v;
  search (last 1024 q) attend to all 384; softmax(QK^T * 768^-0.5);
  out = concat @ w_proj.T + b_proj

Design (fp8e4 DoubleRow everywhere the error budget allows):
  - Host pre-transposes + zero-pads x into fp8 [128ch, G, 1804] per batch
    (per g: padded 18x18 tpl | 18x18 onl | 34x34 srch). No on-device
    transpose or cast.
  - All convs (q stride-1, k/v stride-2) run on PE as diagonal matmuls
    over the 9 taps; taps are paired into fp8 DoubleRow matmuls (2 k-tiles
    per instruction, 0.5 cyc/row).
  - Projections (Q/K/V/out) are fp8 DoubleRow over channel-chunk pairs.
  - Scores are fp8 DoubleRow over head-dim halves: Q/K weights are
    column-permuted host-side so each head's two 32-wide halves live on
    the same 32 partitions at different free offsets.
  - A@V stays bf16 (exp output near 1.0 would lose the attention signal
    in fp8).
  - Softmax denominator via 64 ones columns (value 1/32) in the V
    stationary; reciprocal+apply on DVE; PSUM evacuations spread over
    ACT (cq, qt+bias, exp) / Pool-gpsimd (ckv, kt, v) / DVE (out-proj
    with fused final bias).
  - Scale management for fp8: conv weights x4, w* x64, q/k x8, v x8,
    xatt x256 (ones=1/32 folds the 32 into the reciprocal), out-proj
    descale 1/16384 at evacuation.
  - Output stored bf16, upcast host-side.
"""
import numpy as np
import ml_dtypes

import concourse.bass as bass
import concourse.tile as tile
from concourse import bacc, mybir
from concourse.bass_utils import run_bass_kernel_spmd

F32 = mybir.dt.float32
BF16 = mybir.dt.bfloat16
F8 = mybir.dt.float8e4
AF = mybir.ActivationFunctionType
OP = mybir.AluOpType
PM = mybir.MatmulPerfMode.DoubleRow

EPS = 1e-5
NB = 4          # batches per core
L = 1536
D = 768
G = 6           # channel chunks of 128
NH = 12
HD = 64
SCALE = float(D) ** -0.5
LKV = 384

# padded per-g image layout inside xt plane 0: [18x18 tpl][18x18 onl][34x34 srch]
# plane 1 holds the same data parity-split per image: 4 sub-images
# X_pq(i,j) = P(2i+p, 2j+q), sizes 4x(9x9) / 4x(17x17) -- turns the
# stride-2 conv taps into stride-1 flat windows (DoubleRow needs a
# contiguous innermost window).
PT = 18 * 18            # 324
PS = 34 * 34            # 1156
PG = 2 * PT + PS        # 1804
IMG_BASE = (0, PT, 2 * PT)

# fp8 scale plan
S_CONV = 4.0            # folded into conv weights (host)
S_W = 64.0              # folded into all projection weights (host)
S_QK = 8.0              # qt/kt stored as 8*q (evac scale 1/32)
S_EVAC_QK = S_QK / (S_CONV * S_W)       # = 1/32
S_ONES = 1.0 / 32.0     # ones value in V stationary
S_EVAC_V = 2.0          # V-proj evac: psum = 4*v (bf16 path) -> v_sb = 8*v
S_OUT = 1.0 / 256.0     # out-proj evac descale (xatt = 256*AV, wp raw bf16)

TAPS = [(dh, dw) for dh in range(3) for dw in range(3)]
TAP_PAIRS = [(0, 1), (2, 3), (4, 5), (6, 7)]  # + single tap 8


def _rect(tile_ap, base, dims):
    """AP at tile's partition dim + given free-dim [step,count] list, at free offset base."""
    return bass.AP(tensor=tile_ap.tensor, offset=tile_ap.offset + base,
                   ap=[list(tile_ap.ap[0])] + [list(d) for d in dims])


def build_program(skip=(), nb=NB, reps=1, debug=False):
    nc = bacc.Bacc("TRN2", target_bir_lowering=False, debug=False, num_devices=8)

    xt_d = nc.dram_tensor("xt8", [nb, 128, 2 * G * PG], F8, kind="ExternalInput").ap()
    w_d = {n: nc.dram_tensor(n, [128, G * D], F8 if n in ("wq", "wk") else BF16,
                             kind="ExternalInput").ap()
           for n in ("wq", "wk", "wv", "wp")}
    dg_d = nc.dram_tensor("dg8", [128, 4 * G * 9 * 128], F8, kind="ExternalInput").ap()
    bq_d = nc.dram_tensor("bqr", [128, 8], F32, kind="ExternalInput").ap()
    bfin_d = nc.dram_tensor("bfin_rep", [128, D], F32, kind="ExternalInput").ap()
    out_d = nc.dram_tensor("out", [nb, L, D], BF16, kind="ExternalOutput").ap()
    dbg_d = {}
    if debug:
        for n, shape in (("cq", [G * 128, L]), ("ckv", [2 * G * 128, LKV]),
                         ("kt", [128, 8 * LKV]), ("qt", [128, 8 * 512]),
                         ("v", [128, 3 * NH * 128]), ("xatt", [G * 128, 512])):
            dbg_d[n] = nc.dram_tensor(f"dbg_{n}", shape, F32,
                                      kind="ExternalOutput").ap()

    with tile.TileContext(nc) as tc:
        with (
            tc.tile_pool(name="consts", bufs=1) as consts,
            tc.tile_pool(name="act", bufs=1) as actp,
            tc.tile_pool(name="roll", bufs=1) as roll,
            tc.tile_pool(name="ps_pj", bufs=1, space="PSUM") as ps_pj,
            tc.tile_pool(name="ps_sps", bufs=1, space="PSUM") as ps_sps,
            tc.tile_pool(name="ps_u", bufs=1, space="PSUM") as ps_u,
            tc.tile_pool(name="dbgp", bufs=1) as dbgp,
        ):
            # ---------------- constants ----------------
            w_sb = {}
            for n in ("wq", "wk", "wv", "wp"):
                w_sb[n] = consts.tile([128, G, D],
                                      F8 if n in ("wq", "wk") else BF16,
                                      name=f"{n}_sb")
                nc.sync.dma_start(out=w_sb[n], in_=w_d[n])
            dg_sb = consts.tile([128, 4, G, 9, 128], F8, name="dg_sb")
            nc.sync.dma_start(out=dg_sb, in_=dg_d)
            bq_sb = consts.tile([128, 8], F32, name="bq_sb")
            nc.sync.dma_start(out=bq_sb, in_=bq_d)
            bfin_sb = consts.tile([128, D], F32, name="bfin_sb")
            nc.sync.dma_start(out=bfin_sb, in_=bfin_d)

            # ---------- conv helpers: 9 taps as 4 DR pairs + 1 single ----------
            # Flat contiguous windows (DoubleRow moving APs must be 3D
            # [part, 2, N]); junk accumulates only in pad/overrun columns
            # of PSUM which the evacuation never reads.
            def conv_flat(pj, cidx, g, xt, offs, win, out_base, first, last):
                """offs[ti] = xt free offset of tap ti's flat window (len win).
                Accumulates into pj cols [out_base, out_base+win)."""
                oa = _rect(pj, out_base, [[1, win]])
                mms = []
                for ta, tb in TAP_PAIRS:
                    ia = bass.AP(
                        tensor=xt.tensor, offset=xt.offset + offs[ta],
                        ap=[list(xt.ap[0]), [offs[tb] - offs[ta], 2], [1, win]])
                    mms.append((dg_sb[:, cidx, g, ta:tb + 1, :], ia, PM))
                ia8 = bass.AP(tensor=xt.tensor, offset=xt.offset + offs[8],
                              ap=[list(xt.ap[0]), [1, win]])
                mms.append((dg_sb[:, cidx, g, 8, :], ia8, None))
                for mi, (lw, ia, pm) in enumerate(mms):
                    nc.tensor.matmul(oa, lw, ia, start=(first and mi == 0),
                                     stop=(last and mi == len(mms) - 1),
                                     perf_mode=pm, skip_group_check=True)

            def q_offs(base, W, row0):
                """stride-1 taps: window starts at padded (row0+dh, dw)."""
                return [base + (row0 + dh) * W + dw for dh, dw in TAPS]

            def conv_flat_v(pj, g, xt, offs, win, out_base, first, last):
                """conv-v: 9 taps, each an fp8 DoubleRow pair over the
                residual-quantized diag (sets 2=hi, 3=lo) with a step-0
                k-tile on the moving window (same window read twice)."""
                oa = _rect(pj, out_base, [[1, win]])
                for ti in range(9):
                    ia = bass.AP(
                        tensor=xt.tensor, offset=xt.offset + offs[ti],
                        ap=[list(xt.ap[0]), [0, 2], [1, win]])
                    lw = dg_sb[:, 2:4, g, ti, :]
                    nc.tensor.matmul(oa, lw, ia, start=(first and ti == 0),
                                     stop=(last and ti == 8),
                                     perf_mode=PM, skip_group_check=True)

            def kv_offs(base, S):
                """stride-2 taps on the parity plane: sub-image (dh%2, dw%2)
                at (dh//2, dw//2). S = parity sub-image side (9 or 17)."""
                return [base + (2 * (dh % 2) + dw % 2) * S * S
                        + (dh // 2) * S + dw // 2 for dh, dw in TAPS]

            # ---------------- per batch ----------------
            for rep_i, b in enumerate(bb for _ in range(reps) for bb in range(nb)):
                rb = f"{rep_i}"
                xt = actp.tile([128, 2, G, PG], F8, name=f"xt_{rb}", tag="xt",
                               bufs=2)
                nc.sync.dma_start(out=xt, in_=xt_d[b])

                # ---- conv k/v (stride 2, parity plane)
                #      k -> ckv8 fp8 [128, g, 384]; v -> ckv16 bf16 (V path
                #      carries the output error budget; K path is exp-damped)
                ckv8 = actp.tile([128, G, LKV], F8, name=f"ckv8_{rb}", tag="ckv8",
                                 bufs=2)
                ckv16 = actp.tile([128, G, LKV], BF16, name=f"ckv16_{rb}",
                                  tag="ckv16", bufs=2)
                for ci in range(2):
                    for g in range(6):
                        pj = ps_pj.tile([128, 512], F32, name=f"kv_{rb}_{ci}_{g}",
                                        tag="pj", bufs=3)
                        pb = (G + g) * PG
                        if ci == 0:
                            conv_flat(pj, 1, g, xt, kv_offs(pb, 9), 72,
                                      0, True, False)
                            conv_flat(pj, 1, g, xt, kv_offs(pb + PT, 9), 72,
                                      72, False, False)
                            conv_flat(pj, 1, g, xt, kv_offs(pb + 2 * PT, 17), 272,
                                      144, False, True)
                        else:
                            conv_flat_v(pj, g, xt, kv_offs(pb, 9), 72,
                                        0, True, False)
                            conv_flat_v(pj, g, xt, kv_offs(pb + PT, 9), 72,
                                        72, False, False)
                            conv_flat_v(pj, g, xt, kv_offs(pb + 2 * PT, 17), 272,
                                        144, False, True)
                        ckv = ckv8 if ci == 0 else ckv16
                        cb = g * LKV
                        nc.scalar.copy(
                            _rect(ckv, cb, [[64, 2], [8, 8], [1, 8]]),
                            _rect(pj, 0, [[72, 2], [9, 8], [1, 8]]))
                        nc.scalar.copy(
                            _rect(ckv, cb + 128, [[16, 16], [1, 16]]),
                            _rect(pj, 144, [[17, 16], [1, 16]]))

                # ---- K projection -> kt fp8 [96, grp(4), half(2), 384], = 8*k
                #      (3 heads x 32 hd-half per 96-partition block; matmul
                #      operand base partitions must be 0/32/64)
                kt = actp.tile([96, 4, 2, LKV], F8, name=f"kt_{rb}", tag="kt",
                               bufs=2)
                for jb in range(8):
                    pj = ps_pj.tile([128, 512], F32, name=f"kp_{rb}_{jb}",
                                    tag="pj", bufs=3)
                    for gi, g in enumerate((0, 2, 4)):
                        nc.tensor.matmul(
                            pj[0:96, 0:LKV], w_sb["wk"][:, g:g + 2, jb * 96:(jb + 1) * 96],
                            ckv8[:, g:g + 2, :], start=(gi == 0), stop=(gi == 2),
                            perf_mode=PM)
                    nc.vector.tensor_scalar_mul(kt[:, jb // 2, jb % 2, :],
                                                 pj[0:96, 0:LKV], S_EVAC_QK)

                # ---- V projection -> v_sb bf16 [128, kc(3), nh(12), 128], = 8*v
                #      cols 64:128 of each head hold ones = 1/32 (denominator)
                v_sb = actp.tile([128, 3, NH, 128], BF16, name=f"v_{rb}", tag="v",
                                 bufs=2)
                if rep_i < 2:
                    nc.gpsimd.memset(
                        _rect(v_sb, HD, [[128, 3 * NH], [1, HD]]), S_ONES)
                for mt in range(3):
                    for nh in range(2):
                        pj = ps_pj.tile([128, 512], F32, name=f"vp_{rb}_{mt}_{nh}",
                                        tag="pj", bufs=3)
                        for g in range(6):
                            nc.tensor.matmul(
                                pj[:, 0:LKV], ckv16[:, g, mt * 128:(mt + 1) * 128],
                                w_sb["wv"][:, g, nh * 384:(nh + 1) * 384],
                                start=(g == 0), stop=(g == 5))
                        oa = _rect(v_sb, mt * NH * 128 + nh * 6 * 128,
                                   [[128, 6], [1, HD]])
                        ia = _rect(pj, 0, [[HD, 6], [1, HD]])
                        nc.vector.tensor_scalar_mul(oa, ia, S_EVAC_V)

                # ---- conv-q for one 512-token chunk t, all 6 groups
                cq = actp.tile([128, G, L], F8, name=f"cq_{rb}", tag="cq", bufs=2)

                def convq_g(t, g):
                    gb = g * PG
                    cqb = g * L + t * 512
                    for half in range(2):
                        pj = ps_pj.tile([128, 512], F32,
                                        name=f"cq_{rb}_{t}_{g}_{half}",
                                        tag="pj", bufs=3)
                        if t == 0:
                            offs = q_offs(gb + IMG_BASE[half], 18, 0)
                            conv_flat(pj, 0, g, xt, offs, 288, 0, True, True)
                            nc.scalar.copy(
                                _rect(cq, cqb + half * 256, [[16, 16], [1, 16]]),
                                _rect(pj, 0, [[18, 16], [1, 16]]))
                        else:
                            offs = q_offs(gb + IMG_BASE[2], 34,
                                          16 * (t - 1) + 8 * half)
                            conv_flat(pj, 0, g, xt, offs, 272, 0, True, True)
                            nc.scalar.copy(
                                _rect(cq, cqb + half * 256, [[32, 8], [1, 32]]),
                                _rect(pj, 0, [[34, 8], [1, 32]]))

                # ---- Q projection block jb for chunk t -> qt = 8*q + 8*bq
                def qproj_jb(t, qt, jb):
                    pj = ps_pj.tile([128, 512], F32, name=f"qp_{rb}_{t}_{jb}",
                                    tag="pj", bufs=3)
                    for gi, g in enumerate((0, 2, 4)):
                        nc.tensor.matmul(
                            pj[0:96, :], w_sb["wq"][:, g:g + 2, jb * 96:(jb + 1) * 96],
                            cq[:, g:g + 2, t * 512:(t + 1) * 512],
                            start=(gi == 0), stop=(gi == 2), perf_mode=PM)
                    nc.scalar.activation(qt[:, jb // 2, jb % 2, :], pj[0:96, :],
                                         AF.Identity, bias=bq_sb[0:96, jb:jb + 1],
                                         scale=S_EVAC_QK)

                for g in range(6):
                    convq_g(0, g)
                qt_t = [None, None, None]
                qt_t[0] = roll.tile([96, 4, 2, 512], F8, name=f"qt_{rb}_0",
                                    tag="qt", bufs=2)
                for jb in range(8):
                    qproj_jb(0, qt_t[0], jb)

                # ---- attention per chunk t, with next-chunk conv/proj
                #      interleaved to keep PE fed while ACT runs exp
                xatt = actp.tile([128, G, 512], BF16, name=f"xatt_{rb}",
                                 tag="xatt", bufs=2)
                for t in range(3):
                    qt = qt_t[t]
                    kcs = (0,) if t == 0 else (0, 1, 2)
                    for i in range(6):
                        ha, hb = 2 * i, 2 * i + 1
                        aTs = []
                        for kc in kcs:
                            sps = ps_sps.tile([128, 1024], F32,
                                              name=f"s_{rb}_{t}_{i}_{kc}",
                                              tag="sps", bufs=1)
                            for j, h in enumerate((ha, hb)):
                                grp, p0 = h // 3, 32 * (h % 3)
                                nc.tensor.matmul(
                                    sps[:, j * 512:(j + 1) * 512],
                                    kt[p0:p0 + 32, grp, :, kc * 128:(kc + 1) * 128],
                                    qt[p0:p0 + 32, grp, :, :],
                                    start=True, stop=True, perf_mode=PM,
                                    skip_group_check=True)
                            aT = roll.tile([128, 1024], BF16,
                                           name=f"aT_{rb}_{t}_{i}_{kc}",
                                           tag="aT", bufs=4)
                            nc.scalar.activation(aT, sps, AF.Exp,
                                                 scale=SCALE / (S_QK * S_QK))
                            aTs.append(aT)
                        if t < 2:
                            convq_g(t + 1, i)
                        ups = [ps_u.tile([128, 512], F32, name=f"u_{rb}_{t}_{i}_{j}",
                                         tag="u", bufs=2) for j in range(2)]
                        for j, h in enumerate((ha, hb)):
                            for ki, kc in enumerate(kcs):
                                nc.tensor.matmul(
                                    ups[j], v_sb[:, kc, h, :],
                                    aTs[ki][:, j * 512:(j + 1) * 512],
                                    start=(ki == 0), stop=(ki == len(kcs) - 1))
                        for j in range(2):
                            recip = roll.tile([HD, 512], F32,
                                              name=f"rc_{rb}_{t}_{i}_{j}",
                                              tag="recip", bufs=3)
                            nc.vector.reciprocal(recip, ups[j][HD:2 * HD, :])
                            nc.vector.tensor_mul(
                                xatt[j * HD:(j + 1) * HD, i, :],
                                ups[j][0:HD, :], recip)

                    # ---- output projection + fused bias (DVE) + store
                    if t < 2:
                        qt_t[t + 1] = roll.tile([96, 4, 2, 512], F8,
                                                name=f"qt_{rb}_{t + 1}",
                                                tag="qt", bufs=2)
                    for mt2 in range(4):
                        onat = roll.tile([128, D], BF16, name=f"on_{rb}_{t}_{mt2}",
                                         tag="onat", bufs=3)
                        for nh in range(2):
                            pj = ps_pj.tile([128, 512], F32,
                                            name=f"op_{rb}_{t}_{mt2}_{nh}",
                                            tag="pj", bufs=3)
                            for g in range(6):
                                nc.tensor.matmul(
                                    pj[:, 0:384], xatt[:, g, mt2 * 128:(mt2 + 1) * 128],
                                    w_sb["wp"][:, g, nh * 384:(nh + 1) * 384],
                                    start=(g == 0), stop=(g == 5))
                            nc.vector.scalar_tensor_tensor(
                                out=onat[:, nh * 384:(nh + 1) * 384],
                                in0=pj[:, 0:384], scalar=S_OUT,
                                in1=bfin_sb[:, nh * 384:(nh + 1) * 384],
                                op0=OP.mult, op1=OP.add)
                        if t < 2:
                            for jb in (2 * mt2, 2 * mt2 + 1):
                                qproj_jb(t + 1, qt_t[t + 1], jb)
                        tok0 = t * 512 + mt2 * 128
                        nc.sync.dma_start(out=out_d[b, tok0:tok0 + 128, :], in_=onat)

                if debug and rep_i == 0:
                    def dump(dst, src_ap, rows, cols, di):
                        st = dbgp.tile([128, 1536], F32, name=f"dbg_{di}",
                                       tag="dbg", bufs=2)
                        nc.scalar.copy(st[0:rows, 0:cols], src_ap)
                        nc.sync.dma_start(out=dst, in_=st[0:rows, 0:cols])
                    di = 0
                    for g in range(6):
                        dump(dbg_d["cq"][g * 128:(g + 1) * 128, :],
                             cq[:, g, :], 128, L, di); di += 1
                        dump(dbg_d["xatt"][g * 128:(g + 1) * 128, :],
                             xatt[:, g, :], 128, 512, di); di += 1
                        for ci in range(2):
                            dump(dbg_d["ckv"][(ci * G + g) * 128:(ci * G + g + 1) * 128, :],
                                 ckv[:, ci, g, :], 128, LKV, di); di += 1
                    for jb in range(8):
                        dump(dbg_d["kt"][0:96, jb * LKV:(jb + 1) * LKV],
                             kt[:, jb // 2, jb % 2, :], 96, LKV, di); di += 1
                        dump(dbg_d["qt"][0:96, jb * 512:(jb + 1) * 512],
                             qt_t[2][:, jb // 2, jb % 2, :], 96, 512, di); di += 1
                    for kc in range(3):
                        dump(dbg_d["v"][:, kc * 1536:(kc + 1) * 1536],
                             v_sb[:, kc, :, :], 128, 1536, di); di += 1

    nc.compile()
    return nc


_NC_CACHE = {}


def _get_program():
    if "nc" not in _NC_CACHE:
        _NC_CACHE["nc"] = build_program()
    return _NC_CACHE["nc"]


def _perm_cols():
    """Column permutation for wq/wk so head h's hd halves share 32 partitions
    at matmul-legal bases (0/32/64): 8 blocks of 96 cols, block jb =
    grp*2 + half (grp = h//3), block col p -> orig col
    (3*grp + p//32)*64 + half*32 + p%32."""
    perm = np.zeros(D, dtype=np.int64)
    for jb in range(8):
        grp, half = jb // 2, jb % 2
        for p in range(96):
            perm[jb * 96 + p] = (3 * grp + p // 32) * 64 + half * 32 + p % 32
    return perm


def _pad_images(xg):
    """xg: [..., L] -> (padded [..., PG], parity-split [..., PG])."""
    lead = xg.shape[:-1]
    pad = np.zeros(lead + (PG,), dtype=xg.dtype)
    par = np.zeros(lead + (PG,), dtype=xg.dtype)
    tpl = xg[..., 0:256].reshape(lead + (16, 16))
    onl = xg[..., 256:512].reshape(lead + (16, 16))
    srch = xg[..., 512:1536].reshape(lead + (32, 32))
    for base, img, h in ((0, tpl, 16), (PT, onl, 16), (2 * PT, srch, 32)):
        w = h + 2
        grid = pad[..., base:base + w * w].reshape(lead + (w, w))
        grid[..., 1:h + 1, 1:w - 1] = img
        s = w // 2
        pgrid = par[..., base:base + w * w].reshape(lead + (4, s, s))
        for p in range(2):
            for q in range(2):
                pgrid[..., 2 * p + q, :, :] = grid[..., p::2, q::2]
    return pad, par


def _host_prep(inputs):
    f = lambda k: np.asarray(inputs[k], dtype=np.float32)
    effs = {}
    for n in ("q", "k", "v"):
        inv = f(f"bn_{n}_g") / np.sqrt(f(f"bn_{n}_v") + EPS)
        beta = f(f"bn_{n}_b") - f(f"bn_{n}_m") * inv
        effs[n] = (inv, beta)
    wq, wk, wv, wp = f("wq"), f("wk"), f("wv"), f("w_proj")
    fp8 = ml_dtypes.float8_e4m3
    perm = _perm_cols()

    w8 = {}
    wq_e = (wq * effs["q"][0][None, :]).T * S_W     # [D_in, D_out]
    wk_e = (wk * effs["k"][0][None, :]).T * S_W
    w8["wq"] = np.ascontiguousarray(wq_e[:, perm].reshape(G, 128, D)
                                    .transpose(1, 0, 2).reshape(128, G * D)).astype(fp8)
    w8["wk"] = np.ascontiguousarray(wk_e[:, perm].reshape(G, 128, D)
                                    .transpose(1, 0, 2).reshape(128, G * D)).astype(fp8)
    bf16 = ml_dtypes.bfloat16
    wv_e = (wv * effs["v"][0][None, :]).T
    w8["wv"] = np.ascontiguousarray(wv_e.reshape(G, 128, D)
                                    .transpose(1, 0, 2).reshape(128, G * D)).astype(bf16)
    w8["wp"] = np.ascontiguousarray(wp.T.reshape(G, 128, D)
                                    .transpose(1, 0, 2).reshape(128, G * D)).astype(bf16)

    bq_eff = (wq @ effs["q"][1]) * S_QK             # qt bias (already x8)
    bqr = np.zeros((128, 8), np.float32)
    bqr[0:96, :] = bq_eff[perm].reshape(8, 96).T
    bv_eff = wv @ effs["v"][1]
    b_fin = f("b_proj") + wp @ bv_eff
    bfin_rep = np.broadcast_to(b_fin[None, :], (128, D)).copy()

    # diag conv weights x S_CONV, fp8, layout [128, 4 sets, G, 9, 128]:
    # sets 0=q, 1=k, 2=v_hi, 3=v_lo (v residual-quantized: hi+lo ~ exact)
    dgf = np.zeros((128, 4, G, 9, 128), fp8)
    eye = np.eye(128, dtype=np.float32)
    for six, key in ((0, "conv_q_w"), (1, "conv_k_w"), (2, "conv_v_w")):
        cw = f(key).reshape(D, 9) * S_CONV          # [768, 9]
        cwg = cw.reshape(G, 128, 9)
        for g in range(G):
            for t in range(9):
                d = eye * cwg[g, :, t][:, None]
                hi = d.astype(fp8)
                dgf[:, six, g, t, :] = hi
                if six == 2:
                    dgf[:, 3, g, t, :] = (d - hi.astype(np.float32)).astype(fp8)
    w8["dg8"] = np.ascontiguousarray(dgf.reshape(128, 4 * G * 9 * 128))
    w8["bqr"] = np.ascontiguousarray(bqr)
    w8["bfin_rep"] = bfin_rep
    return w8


def _prep_x(x):
    """x [B, L, D] f32 -> fp8 [B, 128, 2*G*PG]: plane 0 padded, plane 1 parity."""
    B = x.shape[0]
    xg = x.reshape(B, L, G, 128).transpose(0, 2, 3, 1)   # [B, G, 128, L]
    pad, par = _pad_images(xg)                            # [B, G, 128, PG] x2
    xp = np.stack([pad, par], axis=1)                     # [B, 2, G, 128, PG]
    xp = xp.transpose(0, 3, 1, 2, 4).reshape(B, 128, 2 * G * PG)
    return np.ascontiguousarray(xp).astype(ml_dtypes.float8_e4m3)


def kernel(**inputs):
    x = np.asarray(inputs["x"], dtype=np.float32)
    assert x.shape == (32, L, D), x.shape
    const = _host_prep(inputs)
    x8 = _prep_x(x)
    nc = _get_program()
    in_maps = []
    for c in range(8):
        m = dict(const)
        m["xt8"] = np.ascontiguousarray(x8[c * NB:(c + 1) * NB])
        in_maps.append(m)
    res = run_bass_kernel_spmd(nc, in_maps, list(range(8)))
    out = np.concatenate([np.asarray(res.results[c]["out"]) for c in range(8)],
                         axis=0)
    return out.astype(np.float32)
